# revision 1
# baseline (speedup 1.0000x reference)
"""Trainium2 Bass kernel for CRF negative log-likelihood (nn_CRF).

Strategy (see inline comments):
  - data-parallel over batch: 8 cores x 16 sequences each
  - forward algorithm in the exp domain: X_t = d_t * (E^T X_{t-1}) with
    E = exp(transitions); constant per-step rescale exp(-C0) folded into the
    emission tensor; periodic per-batch renormalization (colsum via ones
    matmul) every RENORM steps keeps fp32 in range.
  - masking via an absorbing-STOP construction: E[STOP,STOP]:=1, active steps
    emit d[STOP]=0, frozen steps emit d=onehot(STOP).  The final answer for
    every sequence is X_final[STOP] after one virtual terminal step, so the
    variable lengths never require per-step blending.
  - latency: the 256-step serial chain is split into a forward half
    (t=0..128) and an independent backward half (beta from t=256 down to 129);
    the two chains interleave on PE/DVE and the answer is the per-column dot
    product of the halves at the midpoint.
  - gold path score via one-hot is_equal tensors (GPSIMD) contracted with
    feats (DVE) and pair/end counts (PE matmuls against the one-hots).
  - device emits small per-core partials (raw renorm scales, midpoint dot,
    gold partial sums); the host does the final log/sum ("all-reduce").
"""

import numpy as np

TAG = 52
START, STOP = TAG - 2, TAG - 1
B, S = 128, 256
NCORES = 8
BL = B // NCORES            # 16 sequences per core
MID = 128                   # forward/backward split point
RENORM = 64                 # renormalize every this many steps
C0 = 4.9                    # constant per-step rescale (nats)
MGATE = 64.0                # mask gate constant (exp(-64) == 0 in fp32)
CHUNK = 64                  # emission build chunk (along t)
M32 = (S * BL) // 128       # 32 free columns for (128, M32) gold layout

_CACHE: dict = {}


def _build_nc(debug: bool = False):
    import os
    parts = os.environ.get("KPARTS", "all")   # all | scan | gold | setup
    do_scan = parts in ("all", "scan")
    do_gold = parts in ("all", "gold")
    import concourse.bass as bass
    import concourse.mybir as mybir
    import concourse.tile as tile
    from concourse import bacc

    f32 = mybir.dt.float32
    nc = bacc.Bacc("TRN2", target_bir_lowering=False, debug=debug)

    # ---- external inputs (per-core shards, host-marshalled layouts) ----
    featsT = nc.dram_tensor("featsT", (TAG, S, BL), f32, kind="ExternalInput")
    featsN = nc.dram_tensor("featsN", (128, M32, TAG), f32, kind="ExternalInput")
    mtb = nc.dram_tensor("mtb", (S, BL), f32, kind="ExternalInput")
    maskf = nc.dram_tensor("maskf", (128, M32), f32, kind="ExternalInput")
    mnextf = nc.dram_tensor("mnextf", (128, M32), f32, kind="ExternalInput")
    tagsf = nc.dram_tensor("tagsf", (128, M32), f32, kind="ExternalInput")
    prevf = nc.dram_tensor("prevf", (128, M32), f32, kind="ExternalInput")
    transr = nc.dram_tensor("transr", (TAG, TAG), f32, kind="ExternalInput")
    iotaf = nc.dram_tensor("iotaf", (128, TAG), f32, kind="ExternalInput")
    ident = nc.dram_tensor("ident", (TAG, TAG), f32, kind="ExternalInput")
    colconsts = nc.dram_tensor("colconsts", (TAG, 2), f32, kind="ExternalInput")

    # ---- external outputs ----
    # out_scan column blocks of BL: 0=Ssum, 1..4 = renorm scales
    out_scan = nc.dram_tensor("out_scan", (1, 8 * BL), f32, kind="ExternalOutput")
    # out_gold cols: 0 = per-(b,s)-row emit partials, 1 = trans*count partials,
    #                2 = end-transition partials
    out_gold = nc.dram_tensor("out_gold", (128, 4), f32, kind="ExternalOutput")

    AL = mybir.AluOpType

    with tile.TileContext(nc) as tc:
        with (
            tc.tile_pool(name="persist", bufs=1) as persist,
            tc.tile_pool(name="chunks", bufs=2) as chunks,
            tc.tile_pool(name="state", bufs=3) as statep,
            tc.tile_pool(name="small", bufs=2) as small,
            tc.tile_pool(name="gold", bufs=1) as goldp,
            tc.tile_pool(name="psum", bufs=1, space="PSUM") as psum,
            tc.tile_pool(name="psumg", bufs=1, space="PSUM") as psumg,
        ):
            # ================= constants / setup =================
            tr_sb = persist.tile([TAG, TAG], f32, name="tr_sb", tag="tr_sb")
            nc.sync.dma_start(out=tr_sb, in_=transr[:, :])
            id_sb = persist.tile([TAG, TAG], f32, name="id_sb", tag="id_sb")
            nc.sync.dma_start(out=id_sb, in_=ident[:, :])

            ones_col = persist.tile([TAG, 1], f32, name="ones_col", tag="ones_col")
            nc.vector.memset(ones_col, 1.0)
            ones_row = persist.tile([1, TAG], f32, name="ones_row", tag="ones_row")
            nc.vector.memset(ones_row, 1.0)
            colc = persist.tile([TAG, 2], f32, name="colc", tag="colc")
            nc.sync.dma_start(out=colc, in_=colconsts[:, :])
            sgate = colc[:, 0:1]
            biasc = colc[:, 1:2]

            # patch trans[STOP, STOP] = 0 (so exp gives 1) via tiny const DMA
            # (ident[0, 1] == 0.0); engines cannot address start partition 51,
            # DMA can.
            nc.sync.dma_start(
                out=tr_sb[STOP : STOP + 1, STOP : STOP + 1], in_=ident[0:1, 1:2]
            )
            # Etil = exp(trans); Etil[STOP, STOP] = exp(0) = 1
            Etil = persist.tile([TAG, TAG], f32, name="Etil", tag="Etil")
            nc.scalar.activation(
                out=Etil, in_=tr_sb, func=mybir.ActivationFunctionType.Exp
            )
            # EtilT = Etil^T (PE transpose through PSUM)
            ps_t = psum.tile([TAG, TAG], f32, name="ps_t", tag="ps_b")
            nc.tensor.transpose(ps_t, Etil, id_sb)
            EtilT = persist.tile([TAG, TAG], f32, name="EtilT", tag="EtilT")
            nc.vector.tensor_copy(EtilT, ps_t)

            # ================= emission tensor D (TAG, S, BL) =================
            if not do_scan:
                nc.vector.memset(stage_scan_dummy___ := None or persist.tile(
                    [1, 8 * BL], f32, name="stage_scan", tag="stage_scan"), 1.0)
                nc.sync.dma_start(out=out_scan[:, :], in_=stage_scan_dummy___)
            if do_scan:
                D = persist.tile([TAG, S, BL], f32, name="D", tag="D")
                # build in t-chunks; order 0,3,1,2 so fwd (chunk0) and bwd (chunk3)
                # can start as early as possible
                for c in (0, 3, 1, 2):
                    t0 = c * CHUNK
                    ft = chunks.tile([TAG, CHUNK, BL], f32, name="ft", tag="ft")
                    nc.sync.dma_start(out=ft, in_=featsT[:, t0 : t0 + CHUNK, :])
                    mrep = chunks.tile([TAG, CHUNK, BL], f32, name="mrep", tag="mrep")
                    src = bass.AP(
                        tensor=mtb,
                        offset=t0 * BL,
                        ap=[[0, TAG], [BL, CHUNK], [1, BL]],
                    )
                    nc.sync.dma_start(out=mrep, in_=src)
                    # ft <- (mrep * sgate) + ft
                    nc.vector.scalar_tensor_tensor(
                        out=ft, in0=mrep, scalar=sgate, in1=ft, op0=AL.mult, op1=AL.add
                    )
                    nc.scalar.activation(
                        out=D[:, t0 : t0 + CHUNK, :],
                        in_=ft,
                        func=mybir.ActivationFunctionType.Exp,
                        bias=biasc,
                    )

                # ================= scan state init =================
                X = statep.tile([TAG, BL], f32, name="X", tag="X")
                nc.vector.tensor_scalar_mul(
                    out=X, in0=D[:, 0, :], scalar1=EtilT[:, START : START + 1]
                )
                BT = statep.tile([TAG, BL], f32, name="BT", tag="BT")
                nc.vector.memset(BT, 1.0)
                nc.vector.tensor_scalar_mul(
                    out=BT, in0=BT, scalar1=Etil[:, STOP : STOP + 1]
                )

                stage_scan = persist.tile(
                    [1, 8 * BL], f32, name="stage_scan", tag="stage_scan"
                )
                nc.vector.memset(stage_scan, 0.0)

                def renorm(V, srow, blockname):
                    """V (TAG, BL) sbuf -> V / colsum(V); stage raw colsum in
                    stage_scan[srow]."""
                    ps_s = psum.tile([1, BL], f32, name=f"ps_s_{blockname}", tag="ps_s")
                    nc.tensor.matmul(ps_s, ones_col, V, start=True, stop=True)
                    nc.vector.tensor_copy(stage_scan[0:1, srow * BL : (srow + 1) * BL], ps_s)
                    rcp = small.tile([1, BL], f32, name=f"rcp_{blockname}", tag="rcp")
                    nc.vector.reciprocal(rcp, ps_s)
                    ps_b = psum.tile([TAG, BL], f32, name=f"ps_b_{blockname}", tag="ps_b")
                    nc.tensor.matmul(ps_b, ones_row, rcp, start=True, stop=True)
                    V2 = statep.tile([TAG, BL], f32, name=f"rn_{blockname}", tag=blockname)
                    nc.vector.tensor_mul(V2, V, ps_b)
                    return V2

                # ================= interleaved fwd/bwd scans =================
                # fwd: X_t = d_t * (Etil^T X_{t-1}),          t = 1..MID
                # bwd: beta_{t-1} = Etil (d_t * beta_t),      t = 255..MID+1
                nren_f = 0
                nren_b = 0
                for k in range(1, MID + 1):
                    # ---- forward step t = k ----
                    ps_f = psum.tile([TAG, BL], f32, name="ps_f", tag="ps_f", bufs=2)
                    nc.tensor.matmul(ps_f, Etil, X, start=True, stop=True)
                    Xn = statep.tile([TAG, BL], f32, name="Xn", tag="X")
                    nc.vector.tensor_mul(Xn, ps_f, D[:, k, :])
                    X = Xn
                    if k % RENORM == 0 or k == MID:
                        if k % RENORM == 0 and k != MID:
                            X = renorm(X, 1, "X")
                            nren_f += 1
                        else:
                            X = renorm(X, 2, "X")
                            nren_f += 1
                    # ---- backward step t = S - k (skip when t <= MID) ----
                    t = S - k
                    if t > MID:
                        bt = statep.tile([TAG, BL], f32, name="bt", tag="BT")
                        nc.vector.tensor_mul(bt, BT, D[:, t, :])
                        ps_bw = psum.tile([TAG, BL], f32, name="ps_bw", tag="ps_bw", bufs=2)
                        nc.tensor.matmul(ps_bw, EtilT, bt, start=True, stop=True)
                        BTn = statep.tile([TAG, BL], f32, name="BTn", tag="BT")
                        nc.vector.tensor_copy(BTn, ps_bw)
                        BT = BTn
                        if k % RENORM == 0:
                            BT = renorm(BT, 3, "BT")
                            nren_b += 1
                        elif t == MID + 1:
                            BT = renorm(BT, 4, "BT")
                            nren_b += 1

                # ================= midpoint combine =================
                P = statep.tile([TAG, BL], f32, name="P", tag="X")
                nc.vector.tensor_mul(P, X, BT)
                ps_c = psum.tile([1, BL], f32, name="ps_c", tag="ps_s")
                nc.tensor.matmul(ps_c, ones_col, P, start=True, stop=True)
                nc.vector.tensor_copy(stage_scan[0:1, 0:BL], ps_c)
                nc.sync.dma_start(out=out_scan[:, :], in_=stage_scan)

            # ================= gold score =================
            if not do_gold:
                gdum = goldp.tile([128, 4], f32, name="stage_gold", tag="stage_gold")
                nc.vector.memset(gdum, 1.0)
                nc.sync.dma_start(out=out_gold[:, :], in_=gdum)
            if do_gold:
                FN = goldp.tile([128, M32, TAG], f32, name="FN", tag="FN")
                nc.sync.dma_start(out=FN, in_=featsN[:, :, :])
                MK = goldp.tile([128, M32], f32, name="MK", tag="MK")
                nc.sync.dma_start(out=MK, in_=maskf[:, :])
                MN = goldp.tile([128, M32], f32, name="MN", tag="MN")
                nc.sync.dma_start(out=MN, in_=mnextf[:, :])
                TGf = goldp.tile([128, M32], f32, name="TGf", tag="TGf")
                nc.sync.dma_start(out=TGf, in_=tagsf[:, :])
                PV = goldp.tile([128, M32], f32, name="PV", tag="PV")
                nc.sync.dma_start(out=PV, in_=prevf[:, :])
                IO = goldp.tile([128, TAG], f32, name="IO", tag="IO")
                nc.sync.dma_start(out=IO, in_=iotaf[:, :])

                # tags_m = (tags + 1) * mask - 1
                TGM = goldp.tile([128, M32], f32, name="TGM", tag="TGM")
                nc.vector.tensor_scalar_add(out=TGM, in0=TGf, scalar1=1.0)
                nc.vector.tensor_mul(TGM, TGM, MK)
                nc.vector.tensor_scalar_add(out=TGM, in0=TGM, scalar1=-1.0)
                # w_last = mask - mask_next
                WL = goldp.tile([128, M32], f32, name="WL", tag="WL")
                nc.vector.tensor_sub(WL, MK, MN)

                def bcast_cmp(out_t, vals):
                    # out[p, m, j] = (vals[p, m] == iota[p, j])
                    v3 = bass.AP(
                        tensor=vals.tensor,
                        offset=vals.offset,
                        ap=[vals.ap[0], vals.ap[1], [0, TAG]],
                    )
                    i3 = bass.AP(
                        tensor=IO.tensor,
                        offset=IO.offset,
                        ap=[IO.ap[0], [0, M32], IO.ap[1]],
                    )
                    nc.vector.tensor_tensor(out=out_t, in0=v3, in1=i3, op=AL.is_equal)

                Y = goldp.tile([128, M32, TAG], f32, name="Y", tag="Y")
                bcast_cmp(Y, TGM)
                YP = goldp.tile([128, M32, TAG], f32, name="YP", tag="YP")
                bcast_cmp(YP, PV)

                stage_gold = goldp.tile([128, 4], f32, name="stage_gold", tag="stage_gold")
                nc.vector.memset(stage_gold, 0.0)

                # emit partials: sum_j (Y * featsN) per (b,s)-row
                scrap = goldp.tile([128, M32, TAG], f32, name="scrap", tag="scrap")
                nc.vector.tensor_mul(scrap, Y, FN)
                nc.vector.tensor_reduce(
                    out=stage_gold[:, 0:1],
                    in_=scrap,
                    axis=mybir.AxisListType.XY,
                    op=AL.add,
                )

                # pair counts: cnt[i, j] = sum_bs YP[bs, i] * Y[bs, j]
                ps_cnt = psumg.tile([TAG, TAG], f32, name="ps_cnt", tag="ps_cnt")
                for m in range(M32):
                    nc.tensor.matmul(
                        ps_cnt,
                        YP[:, m, :],
                        Y[:, m, :],
                        start=(m == 0),
                        stop=(m == M32 - 1),
                    )
                # trans partials: sum_j cnt[i, j] * trans[i, j] per i
                scrap2 = goldp.tile([TAG, TAG], f32, name="scrap2", tag="scrap2")
                nc.vector.tensor_mul(scrap2, ps_cnt, tr_sb)
                nc.vector.tensor_reduce(
                    out=stage_gold[0:TAG, 1:2],
                    in_=scrap2,
                    axis=mybir.AxisListType.X,
                    op=AL.add,
                )

                # end counts: endcnt[j] = sum_bs Y[bs, j] * w_last[bs]
                ps_end = psumg.tile([TAG, 1], f32, name="ps_end", tag="ps_end")
                for m in range(M32):
                    nc.tensor.matmul(
                        ps_end,
                        Y[:, m, :],
                        WL[:, m : m + 1],
                        start=(m == 0),
                        stop=(m == M32 - 1),
                    )
                nc.vector.tensor_mul(
                    stage_gold[0:TAG, 2:3], ps_end, tr_sb[:, STOP : STOP + 1]
                )

                nc.sync.dma_start(out=out_gold[:, :], in_=stage_gold)

    nc.compile()
    return nc


def _prep_core_inputs(feats, transitions, mask, tags, core):
    """Layout-only host marshalling of the core's batch shard."""
    f32 = np.float32
    sl = slice(core * BL, (core + 1) * BL)
    f = np.ascontiguousarray(feats[sl]).astype(f32, copy=False)   # (BL,S,T)
    m = mask[sl].astype(f32)                                      # (BL,S)
    tg = tags[sl].astype(f32)                                     # (BL,S)

    featsT = np.ascontiguousarray(f.transpose(2, 1, 0)).copy()    # (T,S,BL)
    featsT[STOP] = 0.0
    featsN = np.ascontiguousarray(f.reshape(BL * S, TAG)).reshape(128, M32, TAG)
    mtb = np.ascontiguousarray(m.T)                               # (S,BL)
    maskf = m.reshape(128, M32)
    mnext = np.concatenate([m[:, 1:], np.zeros((BL, 1), f32)], axis=1)
    mnextf = mnext.reshape(128, M32)
    tagsf = tg.reshape(128, M32)
    prev = np.concatenate([np.full((BL, 1), START, f32), tg[:, :-1]], axis=1)
    prevf = prev.reshape(128, M32)
    transr = transitions.astype(f32, copy=False)
    iotaf = np.broadcast_to(np.arange(TAG, dtype=f32), (128, TAG)).copy()
    ident = np.eye(TAG, dtype=f32)
    colconsts = np.zeros((TAG, 2), f32)
    colconsts[:, 0] = MGATE
    colconsts[STOP, 0] = -MGATE
    colconsts[:, 1] = -(MGATE + C0)
    colconsts[STOP, 1] = 0.0
    return {
        "featsT": np.ascontiguousarray(featsT),
        "featsN": np.ascontiguousarray(featsN),
        "mtb": mtb,
        "maskf": np.ascontiguousarray(maskf),
        "mnextf": np.ascontiguousarray(mnextf),
        "tagsf": np.ascontiguousarray(tagsf),
        "prevf": np.ascontiguousarray(prevf),
        "transr": np.ascontiguousarray(transr),
        "iotaf": iotaf,
        "ident": ident,
        "colconsts": colconsts,
    }


def _combine(results, mask):
    """Host-side unshard: logs of staged scales + partial sums -> scalar."""
    f32 = np.float32
    lengths = mask.astype(np.int64).sum(axis=1)  # (B,)
    fwd = np.float64(0.0)
    gold = np.float64(0.0)
    for core, res in enumerate(results):
        sc = res["out_scan"].astype(np.float64).reshape(8, BL)
        gl = res["out_gold"].astype(np.float64)      # (128, 4)
        ln = np.log(sc[0]) + np.log(sc[1]) + np.log(sc[2]) + np.log(sc[3]) \
            + np.log(sc[4])
        lens = lengths[core * BL : (core + 1) * BL].astype(np.float64)
        fwd += (ln + C0 * lens).sum()
        gold += gl[:, 0].sum() + gl[0:TAG, 1].sum() + gl[0:TAG, 2].sum()
    return np.asarray(fwd - gold, dtype=f32)[()]


def kernel(feats, transitions, mask, tags):
    feats = np.asarray(feats)
    transitions = np.asarray(transitions)
    mask = np.asarray(mask)
    tags = np.asarray(tags)

    if "nc" not in _CACHE:
        _CACHE["nc"] = _build_nc(debug=False)
    nc = _CACHE["nc"]

    from concourse import bass_utils

    in_maps = [
        _prep_core_inputs(feats, transitions, mask, tags, c) for c in range(NCORES)
    ]
    out = bass_utils.run_bass_kernel_spmd(nc, in_maps, core_ids=list(range(NCORES)))
    return _combine(out.results, mask)



# revision 9
# speedup vs baseline: 1.8440x; 1.8440x over previous
"""Trainium2 Bass kernel for CRF negative log-likelihood (nn_CRF) — v2.

Strategy:
  - data-parallel over batch: 8 cores x 16 sequences each.
  - forward algorithm in the exp domain: the fwd chain (alpha, t=0..127) and
    the bwd chain (beta, t=255..128) are MERGED into one 128-step scan over a
    block-diagonal bf16 stationary G (Etil at rows/cols 0..51, Etil^T at
    64..115; blocks at 0/64 keep engine partition bases 32-aligned).  Each
    step is ONE bf16 matmul (PE) + ONE elementwise multiply (DVE) — the
    serial cross-engine round trip is the latency floor, so everything else
    (emission build, gold score) is placed on ACT/GPSIMD/DMA to stay off the
    critical path.
  - emissions D2 (128, 128, BL) bf16 hold fwd emissions on top rows and
    time-reversed bwd emissions on bottom rows; constant per-step rescale
    exp(-C0) folded in via ACT bias; masking via the absorbing-STOP
    construction (only the bwd half needs the mask gate: lengths >= S/2).
  - one renorm at k=64 keeps fp32/bf16 range; raw colsums staged out, host
    adds the logs back.
  - gold path score: host does the integer prep (one-hot of tags, pair/end
    counts); device does all float math (emission gather via one-hot
    multiply+reduce on GPSIMD, transition contraction on DVE).
  - host combine: logs of staged scales + partial sums -> scalar NLL.
"""

import numpy as np

TAG = 52
START, STOP = TAG - 2, TAG - 1
B, S = 128, 256
NCORES = 8
BL = B // NCORES            # 16 sequences per core
HALF = S // 2               # 128 steps per direction
C0 = 4.9                    # constant per-step rescale (nats)
MGATE = 64.0                # mask gate constant (exp(-64) == 0 in fp32)
CH = 16                     # emission build chunk (along the step axis)
M32 = (S * BL) // 128       # 32 free rows for the (128, M32, TAG) gold layout
ROWB = 64                   # partition offset of the bwd block

_CACHE: dict = {}


def _build_nc(debug: bool = False):
    import concourse.bass as bass
    import concourse.mybir as mybir
    import concourse.tile as tile
    from concourse import bacc

    f32 = mybir.dt.float32
    bf16 = mybir.dt.bfloat16
    AL = mybir.AluOpType
    EXP = mybir.ActivationFunctionType.Exp

    nc = bacc.Bacc("TRN2", target_bir_lowering=False, debug=debug)

    # ---- external inputs (per-core shards, host-marshalled layouts) ----
    featsT2 = nc.dram_tensor("featsT2", (128, HALF, BL), f32, kind="ExternalInput")
    mtb = nc.dram_tensor("mtb", (HALF, BL), f32, kind="ExternalInput")
    feats0 = nc.dram_tensor("feats0", (TAG, BL), f32, kind="ExternalInput")
    transr = nc.dram_tensor("transr", (TAG, TAG), f32, kind="ExternalInput")
    identf = nc.dram_tensor("identf", (TAG, TAG), f32, kind="ExternalInput")
    colcs = nc.dram_tensor("colcs", (128, 2), f32, kind="ExternalInput")
    hsumf = nc.dram_tensor("hsumf", (128, 2), bf16, kind="ExternalInput")
    hbcf = nc.dram_tensor("hbcf", (2, 128), f32, kind="ExternalInput")
    onebl = nc.dram_tensor("onebl", (1, BL), bf16, kind="ExternalInput")
    featsN = nc.dram_tensor("featsN", (128, M32, TAG), f32, kind="ExternalInput")
    yhot = nc.dram_tensor("yhot", (128, M32, TAG), f32, kind="ExternalInput")
    cntp = nc.dram_tensor("cntp", (TAG, TAG), f32, kind="ExternalInput")
    cnte = nc.dram_tensor("cnte", (TAG, 1), f32, kind="ExternalInput")

    # ---- external outputs ----
    # out_scan: [0, 0:BL] = midpoint colsum; [0/1, BL:2BL] = fwd/bwd renorm sums
    out_scan = nc.dram_tensor("out_scan", (2, 3 * BL), f32, kind="ExternalOutput")
    # out_gold cols: 0 = per-(b,s)-row emit partials, 1 = trans*cnt partials,
    #                2 = end-transition partials
    out_gold = nc.dram_tensor("out_gold", (128, 4), f32, kind="ExternalOutput")

    with tile.TileContext(nc) as tc:
        with (
            tc.tile_pool(name="persist", bufs=1) as persist,
            tc.tile_pool(name="chunks", bufs=2) as chunks,
            tc.tile_pool(name="state", bufs=3) as statep,
            tc.tile_pool(name="small", bufs=2) as small,
            tc.tile_pool(name="gold", bufs=1) as goldp,
            tc.tile_pool(name="psum", bufs=1, space="PSUM") as psum,
            tc.tile_pool(name="psumg", bufs=1, space="PSUM") as psumg,
        ):
            # ================= constants / setup =================
            tr_sb = persist.tile([TAG, TAG], f32, name="tr_sb", tag="tr_sb")
            nc.sync.dma_start(out=tr_sb, in_=transr[:, :])
            id_sb = persist.tile([TAG, TAG], f32, name="id_sb", tag="id_sb")
            nc.sync.dma_start(out=id_sb, in_=identf[:, :])
            colc = persist.tile([128, 2], f32, name="colc", tag="colc")
            nc.sync.dma_start(out=colc, in_=colcs[:, :])
            Hsum = persist.tile([128, 2], bf16, name="Hsum", tag="Hsum")
            nc.sync.dma_start(out=Hsum, in_=hsumf[:, :])
            Hbc = persist.tile([2, 128], f32, name="Hbc", tag="Hbc")
            nc.sync.dma_start(out=Hbc, in_=hbcf[:, :])

            # patch trans[STOP, STOP] = 0 (so exp gives 1) via tiny const DMA
            # (identf[0, 1] == 0.0); engines cannot address start partition 51,
            # DMA can.
            nc.sync.dma_start(
                out=tr_sb[STOP : STOP + 1, STOP : STOP + 1], in_=identf[0:1, 1:2]
            )
            Etil = persist.tile([TAG, TAG], f32, name="Etil", tag="Etil")
            nc.scalar.activation(out=Etil, in_=tr_sb, func=EXP)
            ps_t = psum.tile([TAG, TAG], f32, name="ps_t", tag="ps_t")
            nc.tensor.transpose(ps_t, Etil, id_sb)
            EtT = persist.tile([TAG, TAG], f32, name="EtT", tag="EtT")
            nc.vector.tensor_copy(EtT, ps_t)
            Et_bf = persist.tile([TAG, TAG], bf16, name="Et_bf", tag="Et_bf")
            nc.vector.tensor_copy(Et_bf, Etil)
            EtT_bf = persist.tile([TAG, TAG], bf16, name="EtT_bf", tag="EtT_bf")
            nc.vector.tensor_copy(EtT_bf, ps_t)

            # G blockdiag (128, 128) bf16; G2 = shifted EtilT for the tail step
            G = persist.tile([128, 128], bf16, name="G", tag="G")
            nc.vector.memset(G, 0.0)
            nc.sync.dma_start(out=G[0:TAG, 0:TAG], in_=Et_bf)
            nc.sync.dma_start(
                out=G[ROWB : ROWB + TAG, ROWB : ROWB + TAG], in_=EtT_bf
            )
            G2 = persist.tile([128, TAG], bf16, name="G2", tag="G2")
            nc.vector.memset(G2, 0.0)
            nc.sync.dma_start(out=G2[ROWB : ROWB + TAG, :], in_=EtT_bf)

            ones52 = persist.tile([TAG, 1], f32, name="ones52", tag="ones52")
            nc.vector.memset(ones52, 1.0)

            # ================= emission tensor D2 (128, HALF, BL) =================
            D2 = persist.tile([128, HALF, BL], bf16, name="D2", tag="D2")
            nc.vector.memset(D2, 0.0)

            # ================= scan state init =================
            V = statep.tile([128, BL], bf16, name="V0", tag="V")
            nc.vector.memset(V, 0.0)
            f0 = small.tile([TAG, BL], f32, name="f0", tag="f0")
            nc.sync.dma_start(out=f0, in_=feats0[:, :])
            D0 = small.tile([TAG, BL], bf16, name="D0", tag="D0")
            nc.scalar.activation(out=D0, in_=f0, func=EXP, bias=colc[0:TAG, 1:2])
            nc.vector.tensor_scalar_mul(
                out=V[0:TAG, :], in0=D0, scalar1=EtT[:, START : START + 1]
            )
            # bottom init: onehot(STOP) at row ROWB+STOP, via DMA (row 115)
            nc.sync.dma_start(
                out=V[ROWB + STOP : ROWB + STOP + 1, :], in_=onebl[:, :]
            )

            # ================= D2 build (chunks along the step axis) =========
            # chunks 0..1 built up front; 2..7 pipelined inside the scan loop
            # (DVE gate STT fills the scan's dependency-wait gaps).
            def emit_chunk(c):
                s0 = c * CH
                ft = chunks.tile([128, CH, BL], f32, name="ft", tag="ft")
                nc.sync.dma_start(out=ft, in_=featsT2[:, s0 : s0 + CH, :])
                mrep = chunks.tile([128, CH, BL], f32, name="mrep", tag="mrep")
                src = bass.AP(
                    tensor=mtb,
                    offset=s0 * BL,
                    ap=[[0, TAG], [BL, CH], [1, BL]],
                )
                nc.sync.dma_start(out=mrep[ROWB : ROWB + TAG, :, :], in_=src)
                # bottom: ft <- (mrep * sgate) + ft  (mask gate)
                nc.vector.scalar_tensor_tensor(
                    out=ft[ROWB : ROWB + TAG, :, :],
                    in0=mrep[ROWB : ROWB + TAG, :, :],
                    scalar=colc[ROWB : ROWB + TAG, 0:1],
                    in1=ft[ROWB : ROWB + TAG, :, :],
                    op0=AL.mult,
                    op1=AL.add,
                )
                nc.scalar.activation(
                    out=D2[0:TAG, s0 : s0 + CH, :],
                    in_=ft[0:TAG, :, :],
                    func=EXP,
                    bias=colc[0:TAG, 1:2],
                )
                nc.scalar.activation(
                    out=D2[ROWB : ROWB + TAG, s0 : s0 + CH, :],
                    in_=ft[ROWB : ROWB + TAG, :, :],
                    func=EXP,
                    bias=colc[ROWB : ROWB + TAG, 1:2],
                )

            emit_chunk(0)
            emit_chunk(1)

            # ================= interleaved fwd/bwd scan =================
            stage_sc = persist.tile([2, 3 * BL], f32, name="stage_sc", tag="ssc")
            nc.vector.memset(stage_sc, 0.0)

            alpha = None
            for k in range(1, HALF + 1):
                if k % CH == 0 and 2 <= (k // CH) + 1 < HALF // CH:
                    emit_chunk((k // CH) + 1)
                ps = psum.tile([128, BL], f32, name="ps", tag="ps", bufs=2)
                nc.tensor.matmul(ps, G, V, start=True, stop=True)
                Vn = statep.tile([128, BL], bf16, name="Vn", tag="V")
                nc.vector.tensor_tensor(
                    out=Vn, in0=ps, in1=D2[:, k - 1, :], op=AL.mult
                )
                V = Vn
                if k == 64:
                    ps_s = psum.tile([2, BL], f32, name="ps_s", tag="ps_s")
                    nc.tensor.matmul(ps_s, Hsum, V, start=True, stop=True)
                    nc.vector.tensor_copy(stage_sc[:, BL : 2 * BL], ps_s)
                    rcp = small.tile([2, BL], f32, name="rcp", tag="rcp")
                    nc.vector.reciprocal(rcp, ps_s)
                    ps_b = psum.tile([128, BL], f32, name="ps_b", tag="ps_b")
                    nc.tensor.matmul(ps_b, Hbc, rcp, start=True, stop=True)
                    Vr = statep.tile([128, BL], bf16, name="Vr", tag="V")
                    nc.vector.tensor_tensor(out=Vr, in0=ps_b, in1=V, op=AL.mult)
                    V = Vr
                if k == HALF - 1:
                    alpha = V

            # ================= gold inputs + partials (GPSIMD, off-path) =====
            FN = goldp.tile([128, M32, TAG], f32, name="FN", tag="FN")
            nc.sync.dma_start(out=FN, in_=featsN[:, :, :])
            Y = goldp.tile([128, M32, TAG], f32, name="Y", tag="Y")
            nc.sync.dma_start(out=Y, in_=yhot[:, :, :])
            cnt_sb = goldp.tile([TAG, TAG], f32, name="cnt_sb", tag="cnt_sb")
            nc.sync.dma_start(out=cnt_sb, in_=cntp[:, :])
            cnte_sb = goldp.tile([TAG, 1], f32, name="cnte_sb", tag="cnte_sb")
            nc.sync.dma_start(out=cnte_sb, in_=cnte[:, :])

            stage_gold = goldp.tile([128, 4], f32, name="stage_gold", tag="sg")
            nc.gpsimd.memset(stage_gold, 0.0)

            scrap = goldp.tile([128, M32, TAG], f32, name="scrap", tag="scrap")
            nc.gpsimd.tensor_tensor(out=scrap, in0=FN, in1=Y, op=AL.mult)
            nc.gpsimd.tensor_reduce(
                out=stage_gold[0:1, 0:1],
                in_=scrap,
                axis=mybir.AxisListType.XYZWC,
                op=AL.add,
            )

            # ================= tail: Z = alpha_127 . beta_127 =================
            ps_f = psumg.tile([TAG, BL], f32, name="ps_f", tag="ps_f")
            nc.tensor.matmul(ps_f, G2, V, start=True, stop=True)
            P = small.tile([TAG, BL], f32, name="P", tag="P")
            nc.vector.tensor_tensor(out=P, in0=ps_f, in1=alpha[0:TAG, :], op=AL.mult)
            ps_c = psumg.tile([1, BL], f32, name="ps_c", tag="ps_c")
            nc.tensor.matmul(ps_c, ones52, P, start=True, stop=True)
            nc.vector.tensor_copy(stage_sc[0:1, 0:BL], ps_c)
            nc.sync.dma_start(out=out_scan[:, :], in_=stage_sc)

            # ================= gold tail: transition contractions =============
            scrap2 = goldp.tile([TAG, TAG], f32, name="scrap2", tag="scrap2")
            nc.vector.tensor_tensor(out=scrap2, in0=cnt_sb, in1=tr_sb, op=AL.mult)
            nc.vector.tensor_reduce(
                out=stage_gold[0:TAG, 1:2],
                in_=scrap2,
                axis=mybir.AxisListType.X,
                op=AL.add,
            )
            nc.vector.tensor_tensor(
                out=stage_gold[0:TAG, 2:3],
                in0=cnte_sb,
                in1=tr_sb[:, STOP : STOP + 1],
                op=AL.mult,
            )
            nc.sync.dma_start(out=out_gold[:, :], in_=stage_gold)

    nc.compile()
    return nc


def _prep_core_inputs(feats, transitions, mask, tags, core):
    """Host marshalling of the core's batch shard: layout + integer prep."""
    import ml_dtypes

    f32 = np.float32
    bf = ml_dtypes.bfloat16
    sl = slice(core * BL, (core + 1) * BL)
    f = np.ascontiguousarray(feats[sl]).astype(f32, copy=False)   # (BL,S,T)
    m = mask[sl].astype(f32)                                      # (BL,S)
    tg = tags[sl].astype(np.int64)                                # (BL,S)

    fT = f.transpose(2, 1, 0)                                     # (T,S,BL)
    ft2 = np.zeros((128, HALF, BL), f32)
    ft2[0:TAG, 0:127, :] = fT[:, 1:128, :]
    ft2[STOP, 0:127, :] = 0.0
    ft2[0:TAG, 127, :] = -200.0                                   # dead fwd slot
    ft2[ROWB : ROWB + TAG, :, :] = fT[:, 255:127:-1, :]
    ft2[ROWB + STOP, :, :] = 0.0

    mtb = np.ascontiguousarray(m.T[255:127:-1, :])                # (HALF,BL)

    f0 = np.ascontiguousarray(f[:, 0, :].T)                       # (T,BL)
    f0[STOP] = 0.0

    colcs = np.zeros((128, 2), f32)
    colcs[0:TAG, 1] = -C0                                         # top bias
    colcs[STOP, 1] = -MGATE
    colcs[ROWB : ROWB + TAG, 0] = MGATE                           # bottom sgate
    colcs[ROWB + STOP, 0] = -MGATE
    colcs[ROWB : ROWB + TAG, 1] = -(MGATE + C0)                   # bottom bias
    colcs[ROWB + STOP, 1] = 0.0

    hsumf = np.zeros((128, 2), bf)
    hsumf[0:TAG, 0] = 1.0
    hsumf[ROWB : ROWB + TAG, 1] = 1.0
    hbcf = np.zeros((2, 128), f32)
    hbcf[0, 0:TAG] = 1.0
    hbcf[1, ROWB : ROWB + TAG] = 1.0

    featsN = np.ascontiguousarray(f.reshape(BL * S, TAG)).reshape(128, M32, TAG)

    tags_m = np.where(m > 0, tg, -1)                              # (BL,S)
    yhot = (
        (tags_m.reshape(BL * S, 1) == np.arange(TAG)[None, :])
        .astype(f32)
        .reshape(128, M32, TAG)
    )

    prev = np.concatenate([np.full((BL, 1), START, np.int64), tg[:, :-1]], axis=1)
    msk = m > 0
    cntp = np.zeros((TAG, TAG), f32)
    np.add.at(cntp, (prev[msk], tg[msk]), 1.0)
    lengths = m.astype(np.int64).sum(axis=1)
    end_ids = np.take_along_axis(tg, (lengths - 1)[:, None], axis=1)[:, 0]
    cnte = np.zeros((TAG, 1), f32)
    np.add.at(cnte, (end_ids, np.zeros_like(end_ids)), 1.0)

    return {
        "featsT2": ft2,
        "mtb": mtb,
        "feats0": f0,
        "transr": np.ascontiguousarray(transitions.astype(f32, copy=False)),
        "identf": np.eye(TAG, dtype=f32),
        "colcs": colcs,
        "hsumf": hsumf,
        "hbcf": hbcf,
        "onebl": np.ones((1, BL), bf),
        "featsN": np.ascontiguousarray(featsN),
        "yhot": np.ascontiguousarray(yhot),
        "cntp": cntp,
        "cnte": cnte,
    }


def _combine(results, mask):
    """Host-side unshard: logs of staged scales + partial sums -> scalar."""
    lengths = np.asarray(mask).astype(np.int64).sum(axis=1)       # (B,)
    fwd = np.float64(0.0)
    gold = np.float64(0.0)
    for core, res in enumerate(results):
        sc = res["out_scan"].astype(np.float64)                   # (2, 3*BL)
        gl = res["out_gold"].astype(np.float64)                   # (128, 4)
        ln = (
            np.log(sc[0, 0:BL])
            + np.log(sc[0, BL : 2 * BL])
            + np.log(sc[1, BL : 2 * BL])
        )
        lens = lengths[core * BL : (core + 1) * BL].astype(np.float64)
        fwd += (ln + C0 * lens).sum()
        gold += gl[0, 0] + gl[0:TAG, 1].sum() + gl[0:TAG, 2].sum()
    return np.asarray(fwd - gold, dtype=np.float32)[()]


def kernel(feats, transitions, mask, tags):
    feats = np.asarray(feats)
    transitions = np.asarray(transitions)
    mask = np.asarray(mask)
    tags = np.asarray(tags)

    if "nc" not in _CACHE:
        _CACHE["nc"] = _build_nc(debug=False)
    nc = _CACHE["nc"]

    from concourse import bass_utils

    in_maps = [
        _prep_core_inputs(feats, transitions, mask, tags, c) for c in range(NCORES)
    ]
    out = bass_utils.run_bass_kernel_spmd(nc, in_maps, core_ids=list(range(NCORES)))
    return _combine(out.results, mask)


# revision 12
# speedup vs baseline: 2.0837x; 1.1300x over previous
"""Trainium2 Bass kernel for CRF negative log-likelihood (nn_CRF) — v3.

Strategy:
  - data-parallel over batch: 8 cores x 16 sequences each.
  - forward algorithm in the exp domain: the fwd chain (alpha, t=0..127) and
    the bwd chain (beta, t=255..128) are MERGED into one 128-step scan over a
    block-diagonal bf16 stationary G (Etil at rows/cols 0..51, Etil^T at
    64..115; blocks at 0/64 keep engine partition bases 32-aligned).  Each
    step is ONE bf16 matmul (PE) + ONE elementwise multiply (DVE); the
    serial PE->DVE->PE round trip (~435ns) is the latency floor, so all other
    work lives on ACT/GPSIMD/DMA:
      * emissions D2 (128, HALF, BL) bf16: fwd emissions on rows 0..51, the
        time-reversed bwd emissions on rows 64..115; exp(-C0) rescale and the
        absorbing-STOP mask gate folded in via per-partition ACT bias plus
        DMA accumulate (accum_op=add) of host-scaled mask rows — no vector
        engine involvement at all.
      * gold-score emission gather: host-built one-hot * feats on GPSIMD.
      * host does integer prep only (one-hots, pair/end counts); all float
        math on feats/transitions happens on device.
  - one renorm at k=64 bounds fp32/bf16 range; raw colsums staged out and the
    host adds the logs back (plus C0 * length per sequence).
"""

import numpy as np

TAG = 52
START, STOP = TAG - 2, TAG - 1
B, S = 128, 256
NCORES = 8
BL = B // NCORES            # 16 sequences per core
HALF = S // 2               # 128 steps per direction
C0 = 4.9                    # constant per-step rescale (nats)
MGATE = 64.0                # mask gate constant (exp(-64) == 0 in fp32)
M32 = (S * BL) // 128       # 32 free rows for the (128, M32, TAG) gold layout
ROWB = 64                   # partition offset of the bwd block
GROUPS = ((0, 16), (16, 48), (48, HALF))   # emission build chunk groups

# packed "smalls" layout (columns in a single (128, SMW) f32 tensor)
C_TR = 0            # [0:52]   rows 0:52  transitions (STOP,STOP pre-patched)
C_ID = 52           # [52:104] rows 0:52  identity
C_CC = 104          # [104:106]           colcs (sgate unused now, bias)
C_HS = 106          # [106:108]           Hsum pattern (f32 -> bf16 copy)
C_HB = 108          # [108:236] rows 0:2  Hbc
C_F0 = 236          # [236:252] rows 0:52 feats[:, 0, :].T
C_CP = 252          # [252:304] rows 0:52 pair counts
C_CE = 304          # [304:305] rows 0:52 end counts
SMW = 305

_CACHE: dict = {}


def _build_nc(debug: bool = False):
    import concourse.bass as bass
    import concourse.mybir as mybir
    import concourse.tile as tile
    from concourse import bacc

    f32 = mybir.dt.float32
    bf16 = mybir.dt.bfloat16
    AL = mybir.AluOpType
    EXP = mybir.ActivationFunctionType.Exp

    nc = bacc.Bacc("TRN2", target_bir_lowering=False, debug=debug)

    # ---- external inputs (per-core shards, host-marshalled layouts) ----
    featsT2 = nc.dram_tensor("featsT2", (128, HALF, BL), f32, kind="ExternalInput")
    mgate = nc.dram_tensor("mgate", (2, HALF, BL), f32, kind="ExternalInput")
    smalls = nc.dram_tensor("smalls", (128, SMW), f32, kind="ExternalInput")
    onebl = nc.dram_tensor("onebl", (1, BL), bf16, kind="ExternalInput")
    fny = nc.dram_tensor("fny", (128, M32, 2 * TAG), f32, kind="ExternalInput")

    # ---- external outputs ----
    # out_scan: [0, 0:BL] = midpoint colsum; [0/1, BL:2BL] = fwd/bwd renorm sums
    out_scan = nc.dram_tensor("out_scan", (2, 3 * BL), f32, kind="ExternalOutput")
    # out_gold: [0,0] = emit sum; col1 = trans*cnt partials; col2 = end partials
    out_gold = nc.dram_tensor("out_gold", (128, 4), f32, kind="ExternalOutput")

    with tile.TileContext(nc) as tc:
        with (
            tc.tile_pool(name="persist", bufs=1) as persist,
            tc.tile_pool(name="chunks", bufs=1) as chunks,
            tc.tile_pool(name="state", bufs=3) as statep,
            tc.tile_pool(name="small", bufs=2) as small,
            tc.tile_pool(name="gold", bufs=1) as goldp,
            tc.tile_pool(name="psum", bufs=1, space="PSUM") as psum,
            tc.tile_pool(name="psumg", bufs=1, space="PSUM") as psumg,
        ):
            # ================= packed smalls =================
            SM = persist.tile([128, SMW], f32, name="SM", tag="SM")
            nc.sync.dma_start(out=SM, in_=smalls[:, :])
            tr_sb = SM[0:TAG, C_TR : C_TR + TAG]
            id_sb = SM[0:TAG, C_ID : C_ID + TAG]
            colc = SM[:, C_CC : C_CC + 2]
            Hbc = SM[0:2, C_HB : C_HB + 128]

            Hsum = persist.tile([128, 2], bf16, name="Hsum", tag="Hsum")
            nc.vector.tensor_copy(Hsum, SM[:, C_HS : C_HS + 2])

            # ================= transitions -> G blockdiag =================
            Etil = persist.tile([TAG, TAG], f32, name="Etil", tag="Etil")
            nc.scalar.activation(out=Etil, in_=tr_sb, func=EXP)
            ps_t = psum.tile([TAG, TAG], f32, name="ps_t", tag="ps_t")
            nc.tensor.transpose(ps_t, Etil, id_sb)
            EtT = persist.tile([TAG, TAG], f32, name="EtT", tag="EtT")
            nc.vector.tensor_copy(EtT, ps_t)
            Et_bf = persist.tile([TAG, TAG], bf16, name="Et_bf", tag="Et_bf")
            nc.vector.tensor_copy(Et_bf, Etil)
            EtT_bf = persist.tile([TAG, TAG], bf16, name="EtT_bf", tag="EtT_bf")
            nc.vector.tensor_copy(EtT_bf, ps_t)

            G = persist.tile([128, 128], bf16, name="G", tag="G")
            nc.vector.memset(G, 0.0)
            nc.sync.dma_start(out=G[0:TAG, 0:TAG], in_=Et_bf)
            nc.sync.dma_start(
                out=G[ROWB : ROWB + TAG, ROWB : ROWB + TAG], in_=EtT_bf
            )
            G2 = persist.tile([128, TAG], bf16, name="G2", tag="G2")
            nc.vector.memset(G2, 0.0)
            nc.sync.dma_start(out=G2[ROWB : ROWB + TAG, :], in_=EtT_bf)

            ones52 = persist.tile([TAG, 1], f32, name="ones52", tag="ones52")
            nc.vector.memset(ones52, 1.0)

            # ================= scan state init =================
            V = statep.tile([128, BL], bf16, name="V0", tag="V")
            nc.vector.memset(V, 0.0)
            D0 = small.tile([TAG, BL], bf16, name="D0", tag="D0")
            nc.scalar.activation(
                out=D0,
                in_=SM[0:TAG, C_F0 : C_F0 + BL],
                func=EXP,
                bias=colc[0:TAG, 1:2],
            )
            nc.vector.tensor_scalar_mul(
                out=V[0:TAG, :], in0=D0, scalar1=EtT[:, START : START + 1]
            )
            # bottom init: onehot(STOP) at row ROWB+STOP (DMA: arbitrary base)
            nc.sync.dma_start(
                out=V[ROWB + STOP : ROWB + STOP + 1, :], in_=onebl[:, :]
            )

            # ================= emission tensor D2 (128, HALF, BL) ============
            D2 = persist.tile([128, HALF, BL], bf16, name="D2", tag="D2")
            nc.vector.memset(D2, 0.0)

            for s0, s1 in GROUPS:
                n = s1 - s0
                ft = chunks.tile([128, n, BL], f32, name=f"ft{s0}", tag=f"ft{s0}")
                nc.sync.dma_start(out=ft, in_=featsT2[:, s0:s1, :])
                # mask gate via DMA accumulate: rows 64..114 += m*MGATE,
                # row 115 (STOP) += (1-m)*MGATE
                srcp = bass.AP(
                    tensor=mgate,
                    offset=s0 * BL,
                    ap=[[0, TAG - 1], [BL, n], [1, BL]],
                )
                nc.gpsimd.dma_start(
                    out=ft[ROWB : ROWB + TAG - 1, :, :],
                    in_=srcp,
                    accum_op=AL.add,
                )
                srcn = bass.AP(
                    tensor=mgate,
                    offset=HALF * BL + s0 * BL,
                    ap=[[0, 1], [BL, n], [1, BL]],
                )
                nc.gpsimd.dma_start(
                    out=ft[ROWB + TAG - 1 : ROWB + TAG, :, :],
                    in_=srcn,
                    accum_op=AL.add,
                )
                nc.scalar.activation(
                    out=D2[0:TAG, s0:s1, :],
                    in_=ft[0:TAG, :, :],
                    func=EXP,
                    bias=colc[0:TAG, 1:2],
                )
                nc.scalar.activation(
                    out=D2[ROWB : ROWB + TAG, s0:s1, :],
                    in_=ft[ROWB : ROWB + TAG, :, :],
                    func=EXP,
                    bias=colc[ROWB : ROWB + TAG, 1:2],
                )

            # ================= interleaved fwd/bwd scan =================
            stage_sc = persist.tile([2, 3 * BL], f32, name="stage_sc", tag="ssc")
            nc.vector.memset(stage_sc, 0.0)

            alpha = None
            for k in range(1, HALF + 1):
                ps = psum.tile([128, BL], f32, name="ps", tag="ps", bufs=2)
                nc.tensor.matmul(ps, G, V, start=True, stop=True)
                Vn = statep.tile([128, BL], bf16, name="Vn", tag="V")
                nc.vector.tensor_tensor(
                    out=Vn, in0=ps, in1=D2[:, k - 1, :], op=AL.mult
                )
                V = Vn
                if k == 64:
                    ps_s = psum.tile([2, BL], f32, name="ps_s", tag="ps_s")
                    nc.tensor.matmul(ps_s, Hsum, V, start=True, stop=True)
                    nc.vector.tensor_copy(stage_sc[:, BL : 2 * BL], ps_s)
                    rcp = small.tile([2, BL], f32, name="rcp", tag="rcp")
                    nc.vector.reciprocal(rcp, ps_s)
                    ps_b = psum.tile([128, BL], f32, name="ps_b", tag="ps_b")
                    nc.tensor.matmul(ps_b, Hbc, rcp, start=True, stop=True)
                    Vr = statep.tile([128, BL], bf16, name="Vr", tag="V")
                    nc.vector.tensor_tensor(out=Vr, in0=ps_b, in1=V, op=AL.mult)
                    V = Vr
                if k == HALF - 1:
                    alpha = V

            # ================= gold (GPSIMD, off the scan path) ==============
            FNY = goldp.tile([128, M32, 2 * TAG], f32, name="FNY", tag="FNY")
            nc.sync.dma_start(out=FNY, in_=fny[:, :, :])
            stage_gold = goldp.tile([128, 4], f32, name="stage_gold", tag="sg")
            nc.gpsimd.memset(stage_gold, 0.0)
            scrap = goldp.tile([128, M32, TAG], f32, name="scrap", tag="scrap")
            nc.gpsimd.tensor_tensor(
                out=scrap,
                in0=FNY[:, :, 0:TAG],
                in1=FNY[:, :, TAG : 2 * TAG],
                op=AL.mult,
            )
            nc.gpsimd.tensor_reduce(
                out=stage_gold[0:1, 0:1],
                in_=scrap,
                axis=mybir.AxisListType.XYZWC,
                op=AL.add,
            )

            # ================= tail: Z = alpha_127 . beta_127 =================
            ps_f = psumg.tile([TAG, BL], f32, name="ps_f", tag="ps_f")
            nc.tensor.matmul(ps_f, G2, V, start=True, stop=True)
            P = small.tile([TAG, BL], f32, name="P", tag="P")
            nc.vector.tensor_tensor(out=P, in0=ps_f, in1=alpha[0:TAG, :], op=AL.mult)
            ps_c = psumg.tile([1, BL], f32, name="ps_c", tag="ps_c")
            nc.tensor.matmul(ps_c, ones52, P, start=True, stop=True)
            nc.vector.tensor_copy(stage_sc[0:1, 0:BL], ps_c)
            nc.sync.dma_start(out=out_scan[:, :], in_=stage_sc)

            # ================= gold tail: transition contractions =============
            scrap2 = small.tile([TAG, TAG], f32, name="scrap2", tag="scrap2")
            nc.vector.tensor_tensor(
                out=scrap2,
                in0=SM[0:TAG, C_CP : C_CP + TAG],
                in1=tr_sb,
                op=AL.mult,
            )
            nc.vector.tensor_reduce(
                out=stage_gold[0:TAG, 1:2],
                in_=scrap2,
                axis=mybir.AxisListType.X,
                op=AL.add,
            )
            nc.vector.tensor_tensor(
                out=stage_gold[0:TAG, 2:3],
                in0=SM[0:TAG, C_CE : C_CE + 1],
                in1=tr_sb[:, STOP : STOP + 1],
                op=AL.mult,
            )
            nc.sync.dma_start(out=out_gold[:, :], in_=stage_gold)

    nc.compile()
    return nc


def _prep_core_inputs(feats, transitions, mask, tags, core):
    """Host marshalling of the core's batch shard: layout + integer prep."""
    import ml_dtypes

    f32 = np.float32
    bf = ml_dtypes.bfloat16
    sl = slice(core * BL, (core + 1) * BL)
    f = np.ascontiguousarray(feats[sl]).astype(f32, copy=False)   # (BL,S,T)
    m = mask[sl].astype(f32)                                      # (BL,S)
    tg = tags[sl].astype(np.int64)                                # (BL,S)

    fT = f.transpose(2, 1, 0)                                     # (T,S,BL)
    ft2 = np.zeros((128, HALF, BL), f32)
    ft2[0:TAG, 0:127, :] = fT[:, 1:128, :]
    ft2[STOP, 0:127, :] = 0.0
    ft2[0:TAG, 127, :] = -200.0                                   # dead fwd slot
    ft2[ROWB : ROWB + TAG, :, :] = fT[:, 255:127:-1, :]
    ft2[ROWB + STOP, :, :] = 0.0

    mtb = np.ascontiguousarray(m.T[255:127:-1, :])                # (HALF,BL)
    mg = np.stack([mtb * MGATE, (1.0 - mtb) * MGATE]).astype(f32)  # (2,HALF,BL)

    smalls = np.zeros((128, SMW), f32)
    trp = transitions.astype(f32).copy()
    trp[STOP, STOP] = 0.0
    smalls[0:TAG, C_TR : C_TR + TAG] = trp
    smalls[0:TAG, C_ID : C_ID + TAG] = np.eye(TAG, dtype=f32)
    # colcs: col0 unused, col1 = per-partition exp bias
    smalls[0:TAG, C_CC + 1] = -C0
    smalls[STOP, C_CC + 1] = -MGATE
    smalls[ROWB : ROWB + TAG, C_CC + 1] = -(MGATE + C0)
    smalls[ROWB + STOP, C_CC + 1] = -MGATE   # pairs with the (1-m) gate row
    smalls[0:TAG, C_HS] = 1.0                 # Hsum col0: fwd half
    smalls[ROWB : ROWB + TAG, C_HS + 1] = 1.0
    smalls[0, C_HB : C_HB + TAG] = 1.0        # Hbc row0 -> fwd rows
    smalls[1, C_HB + ROWB : C_HB + ROWB + TAG] = 1.0
    f0 = f[:, 0, :].T.copy()
    f0[STOP] = 0.0
    smalls[0:TAG, C_F0 : C_F0 + BL] = f0

    prev = np.concatenate([np.full((BL, 1), START, np.int64), tg[:, :-1]], axis=1)
    msk = m > 0
    cntp = np.zeros((TAG, TAG), f32)
    np.add.at(cntp, (prev[msk], tg[msk]), 1.0)
    smalls[0:TAG, C_CP : C_CP + TAG] = cntp
    lengths = m.astype(np.int64).sum(axis=1)
    end_ids = np.take_along_axis(tg, (lengths - 1)[:, None], axis=1)[:, 0]
    cnte = np.zeros((TAG,), f32)
    np.add.at(cnte, end_ids, 1.0)
    smalls[0:TAG, C_CE] = cnte

    featsN = np.ascontiguousarray(f.reshape(BL * S, TAG)).reshape(128, M32, TAG)
    tags_m = np.where(m > 0, tg, -1)
    yhot = (
        (tags_m.reshape(BL * S, 1) == np.arange(TAG)[None, :])
        .astype(f32)
        .reshape(128, M32, TAG)
    )
    fny = np.concatenate([featsN, yhot], axis=2)                  # (128,M32,104)

    return {
        "featsT2": ft2,
        "mgate": mg,
        "smalls": smalls,
        "onebl": np.ones((1, BL), bf),
        "fny": np.ascontiguousarray(fny),
    }


def _combine(results, mask):
    """Host-side unshard: logs of staged scales + partial sums -> scalar."""
    lengths = np.asarray(mask).astype(np.int64).sum(axis=1)       # (B,)
    fwd = np.float64(0.0)
    gold = np.float64(0.0)
    for core, res in enumerate(results):
        sc = res["out_scan"].astype(np.float64)                   # (2, 3*BL)
        gl = res["out_gold"].astype(np.float64)                   # (128, 4)
        ln = (
            np.log(sc[0, 0:BL])
            + np.log(sc[0, BL : 2 * BL])
            + np.log(sc[1, BL : 2 * BL])
        )
        lens = lengths[core * BL : (core + 1) * BL].astype(np.float64)
        fwd += (ln + C0 * lens).sum()
        gold += gl[0, 0] + gl[0:TAG, 1].sum() + gl[0:TAG, 2].sum()
    return np.asarray(fwd - gold, dtype=np.float32)[()]


def kernel(feats, transitions, mask, tags):
    feats = np.asarray(feats)
    transitions = np.asarray(transitions)
    mask = np.asarray(mask)
    tags = np.asarray(tags)

    if "nc" not in _CACHE:
        _CACHE["nc"] = _build_nc(debug=False)
    nc = _CACHE["nc"]

    from concourse import bass_utils

    in_maps = [
        _prep_core_inputs(feats, transitions, mask, tags, c) for c in range(NCORES)
    ]
    out = bass_utils.run_bass_kernel_spmd(nc, in_maps, core_ids=list(range(NCORES)))
    return _combine(out.results, mask)


# revision 17
# speedup vs baseline: 2.1430x; 1.0285x over previous
"""Trainium2 Bass kernel for CRF negative log-likelihood (nn_CRF) — v3.

Strategy:
  - data-parallel over batch: 8 cores x 16 sequences each.
  - forward algorithm in the exp domain: the fwd chain (alpha, t=0..127) and
    the bwd chain (beta, t=255..128) are MERGED into one 128-step scan over a
    block-diagonal bf16 stationary G (Etil at rows/cols 0..51, Etil^T at
    64..115; blocks at 0/64 keep engine partition bases 32-aligned).  Each
    step is ONE bf16 matmul (PE) + ONE elementwise multiply (DVE); the
    serial PE->DVE->PE round trip (~435ns) is the latency floor, so all other
    work lives on ACT/GPSIMD/DMA:
      * emissions D2 (128, HALF, BL) bf16: fwd emissions on rows 0..51, the
        time-reversed bwd emissions on rows 64..115; exp(-C0) rescale and the
        absorbing-STOP mask gate folded in via per-partition ACT bias plus
        DMA accumulate (accum_op=add) of host-scaled mask rows — no vector
        engine involvement at all.
      * gold-score emission gather: host-built one-hot * feats on GPSIMD.
      * host does integer prep only (one-hots, pair/end counts); all float
        math on feats/transitions happens on device.
  - one renorm at k=64 bounds fp32/bf16 range; raw colsums staged out and the
    host adds the logs back (plus C0 * length per sequence).
"""

import numpy as np

TAG = 52
START, STOP = TAG - 2, TAG - 1
B, S = 128, 256
NCORES = 8
BL = B // NCORES            # 16 sequences per core
HALF = S // 2               # 128 steps per direction
C0 = 4.9                    # constant per-step rescale (nats)
MGATE = 64.0                # mask gate constant (exp(-64) == 0 in fp32)
M32 = (S * BL) // 128       # 32 free rows for the (128, M32, TAG) gold layout
ROWB = 64                   # partition offset of the bwd block
GROUPS = ((0, 16), (16, 48), (48, HALF))   # emission build chunk groups

# packed "smalls" layout (columns in a single (128, SMW) f32 tensor)
C_TR = 0            # [0:52]   rows 0:52  transitions (STOP,STOP pre-patched)
C_TT = 52           # [52:104] rows 0:52  transitions TRANSPOSED (same patch)
C_CC = 104          # [104:106]           colcs (sgate unused now, bias)
C_HS = 106          # [106:108]           Hsum pattern (f32 -> bf16 copy)
C_HB = 108          # [108:236] rows 0:2  Hbc
C_F0 = 236          # [236:252] rows 0:52 feats[:, 0, :].T
C_CP = 252          # [252:304] rows 0:52 pair counts
C_CE = 304          # [304:305] rows 0:52 end counts
SMW = 305

_CACHE: dict = {}


def _build_nc(debug: bool = False):
    import concourse.bass as bass
    import concourse.mybir as mybir
    import concourse.tile as tile
    from concourse import bacc

    f32 = mybir.dt.float32
    bf16 = mybir.dt.bfloat16
    AL = mybir.AluOpType
    EXP = mybir.ActivationFunctionType.Exp

    nc = bacc.Bacc("TRN2", target_bir_lowering=False, debug=debug)

    # ---- external inputs (per-core shards, host-marshalled layouts) ----
    featsT2 = nc.dram_tensor("featsT2", (128, HALF, BL), f32, kind="ExternalInput")
    mgate = nc.dram_tensor("mgate", (2, HALF, BL), f32, kind="ExternalInput")
    smalls = nc.dram_tensor("smalls", (128, SMW), f32, kind="ExternalInput")
    onebl = nc.dram_tensor("onebl", (1, BL), bf16, kind="ExternalInput")
    fny = nc.dram_tensor("fny", (128, M32, 2 * TAG), f32, kind="ExternalInput")

    # ---- external outputs ----
    # out_scan: [0, 0:BL] = midpoint colsum; [0/1, BL:2BL] = fwd/bwd renorm sums
    out_scan = nc.dram_tensor("out_scan", (2, 3 * BL), f32, kind="ExternalOutput")
    # out_gold: [0,0] = emit sum; col1 = trans*cnt partials; col2 = end partials
    out_gold = nc.dram_tensor("out_gold", (128, 4), f32, kind="ExternalOutput")

    with tile.TileContext(nc) as tc:
        with (
            tc.tile_pool(name="persist", bufs=1) as persist,
            tc.tile_pool(name="chunks", bufs=1) as chunks,
            tc.tile_pool(name="state", bufs=3) as statep,
            tc.tile_pool(name="small", bufs=2) as small,
            tc.tile_pool(name="gold", bufs=1) as goldp,
            tc.tile_pool(name="psum", bufs=1, space="PSUM") as psum,
            tc.tile_pool(name="psumg", bufs=1, space="PSUM") as psumg,
        ):
            # ---- ACT activation-table prefetch: dummy exp at t=0 ----
            junk = small.tile([1, 1], f32, name="junk", tag="junk")
            nc.gpsimd.memset(junk, 0.0)
            junk2 = small.tile([1, 1], f32, name="junk2", tag="junk2")
            nc.scalar.activation(out=junk2, in_=junk, func=EXP)

            # ---- emission group-0 DMAs first: longest dependency chain ----
            fts = {}
            for s0, s1 in GROUPS:
                fts[s0] = chunks.tile(
                    [128, s1 - s0, BL], f32, name=f"ft{s0}", tag=f"ft{s0}"
                )
            nc.sync.dma_start(out=fts[0], in_=featsT2[:, 0 : GROUPS[0][1], :])

            # ================= packed smalls =================
            SM = persist.tile([128, SMW], f32, name="SM", tag="SM")
            nc.sync.dma_start(out=SM, in_=smalls[:, :])
            tr_sb = SM[0:TAG, C_TR : C_TR + TAG]
            colc = SM[:, C_CC : C_CC + 2]
            Hbc = SM[0:2, C_HB : C_HB + 128]

            def emit_gate(s0, s1):
                # mask gate via DMA accumulate: rows 64..114 += m*MGATE,
                # row 115 (STOP) += (1-m)*MGATE
                n = s1 - s0
                ft = fts[s0]
                srcp = bass.AP(
                    tensor=mgate,
                    offset=s0 * BL,
                    ap=[[0, TAG - 1], [BL, n], [1, BL]],
                )
                nc.gpsimd.dma_start(
                    out=ft[ROWB : ROWB + TAG - 1, :, :], in_=srcp, accum_op=AL.add
                )
                srcn = bass.AP(
                    tensor=mgate,
                    offset=HALF * BL + s0 * BL,
                    ap=[[0, 1], [BL, n], [1, BL]],
                )
                nc.gpsimd.dma_start(
                    out=ft[ROWB + TAG - 1 : ROWB + TAG, :, :],
                    in_=srcn,
                    accum_op=AL.add,
                )

            emit_gate(*GROUPS[0])

            # ================= transitions -> G blockdiag =================
            # one exp over [tr | trT] gives both blocks; no PE transpose
            EtB = persist.tile([TAG, 2 * TAG], f32, name="EtB", tag="EtB")
            nc.scalar.activation(out=EtB, in_=SM[0:TAG, C_TR : C_TR + 2 * TAG], func=EXP)
            EtT = EtB[:, TAG : 2 * TAG]
            EtB_bf = persist.tile([TAG, 2 * TAG], bf16, name="EtB_bf", tag="EtB_bf")
            nc.vector.tensor_copy(EtB_bf, EtB)

            G = persist.tile([128, 128], bf16, name="G", tag="G")
            nc.vector.memset(G, 0.0)
            nc.sync.dma_start(out=G[0:TAG, 0:TAG], in_=EtB_bf[:, 0:TAG])
            nc.sync.dma_start(
                out=G[ROWB : ROWB + TAG, ROWB : ROWB + TAG],
                in_=EtB_bf[:, TAG : 2 * TAG],
            )
            G2 = persist.tile([128, TAG], bf16, name="G2", tag="G2")
            nc.vector.memset(G2, 0.0)
            nc.sync.dma_start(
                out=G2[ROWB : ROWB + TAG, :], in_=EtB_bf[:, TAG : 2 * TAG]
            )

            Hsum = persist.tile([128, 2], bf16, name="Hsum", tag="Hsum")
            nc.vector.tensor_copy(Hsum, SM[:, C_HS : C_HS + 2])
            ones52 = persist.tile([TAG, 1], bf16, name="ones52", tag="ones52")
            nc.vector.memset(ones52, 1.0)

            # ================= scan state init =================
            V = statep.tile([128, BL], bf16, name="V0", tag="V")
            nc.vector.memset(V, 0.0)
            D0 = small.tile([TAG, BL], bf16, name="D0", tag="D0")
            nc.scalar.activation(
                out=D0,
                in_=SM[0:TAG, C_F0 : C_F0 + BL],
                func=EXP,
                bias=colc[0:TAG, 1:2],
            )
            nc.vector.tensor_scalar_mul(
                out=V[0:TAG, :],
                in0=D0,
                scalar1=EtB[0:TAG, TAG + START : TAG + START + 1],
            )
            # bottom init: onehot(STOP) at row ROWB+STOP (DMA: arbitrary base)
            nc.sync.dma_start(
                out=V[ROWB + STOP : ROWB + STOP + 1, :], in_=onebl[:, :]
            )

            # ================= emission tensor D2 (128, HALF, BL) ============
            D2 = persist.tile([128, HALF, BL], bf16, name="D2", tag="D2")
            nc.vector.memset(D2, 0.0)

            def emit_exps(s0, s1):
                ft = fts[s0]
                nc.scalar.activation(
                    out=D2[0:TAG, s0:s1, :],
                    in_=ft[0:TAG, :, :],
                    func=EXP,
                    bias=colc[0:TAG, 1:2],
                )
                nc.scalar.activation(
                    out=D2[ROWB : ROWB + TAG, s0:s1, :],
                    in_=ft[ROWB : ROWB + TAG, :, :],
                    func=EXP,
                    bias=colc[ROWB : ROWB + TAG, 1:2],
                )

            emit_exps(*GROUPS[0])
            for s0, s1 in GROUPS[1:]:
                nc.sync.dma_start(out=fts[s0], in_=featsT2[:, s0:s1, :])
                emit_gate(s0, s1)
                emit_exps(s0, s1)

            # ================= interleaved fwd/bwd scan =================
            stage_sc = persist.tile([2, 3 * BL], f32, name="stage_sc", tag="ssc")
            nc.vector.memset(stage_sc, 0.0)

            alpha = None
            for k in range(1, HALF + 1):
                ps = psum.tile([128, BL], f32, name="ps", tag="ps", bufs=2)
                nc.tensor.matmul(ps, G, V, start=True, stop=True)
                Vn = statep.tile([128, BL], bf16, name="Vn", tag="V")
                nc.vector.tensor_tensor(
                    out=Vn, in0=ps, in1=D2[:, k - 1, :], op=AL.mult
                )
                V = Vn
                if k == 64:
                    ps_s = psum.tile([2, BL], f32, name="ps_s", tag="ps_s")
                    nc.tensor.matmul(ps_s, Hsum, V, start=True, stop=True)
                    nc.vector.tensor_copy(stage_sc[:, BL : 2 * BL], ps_s)
                    rcp = small.tile([2, BL], f32, name="rcp", tag="rcp")
                    nc.vector.reciprocal(rcp, ps_s)
                    ps_b = psum.tile([128, BL], f32, name="ps_b", tag="ps_b")
                    nc.tensor.matmul(ps_b, Hbc, rcp, start=True, stop=True)
                    Vr = statep.tile([128, BL], bf16, name="Vr", tag="V")
                    nc.vector.tensor_tensor(out=Vr, in0=ps_b, in1=V, op=AL.mult)
                    V = Vr
                if k == HALF - 1:
                    alpha = V

            # ================= gold (GPSIMD, off the scan path) ==============
            FNY = goldp.tile([128, M32, 2 * TAG], f32, name="FNY", tag="FNY")
            nc.sync.dma_start(out=FNY, in_=fny[:, :, :])
            stage_gold = goldp.tile([128, 4], f32, name="stage_gold", tag="sg")
            nc.gpsimd.memset(stage_gold, 0.0)
            scrap = goldp.tile([128, M32, TAG], f32, name="scrap", tag="scrap")
            nc.gpsimd.tensor_tensor(
                out=scrap,
                in0=FNY[:, :, 0:TAG],
                in1=FNY[:, :, TAG : 2 * TAG],
                op=AL.mult,
            )
            nc.gpsimd.tensor_reduce(
                out=stage_gold[0:1, 0:1],
                in_=scrap,
                axis=mybir.AxisListType.XYZWC,
                op=AL.add,
            )

            # ================= tail: Z = alpha_127 . beta_127 =================
            ps_f = psumg.tile([TAG, BL], f32, name="ps_f", tag="ps_f")
            nc.tensor.matmul(ps_f, G2, V, start=True, stop=True)
            P = small.tile([TAG, BL], bf16, name="P", tag="P")
            nc.vector.tensor_tensor(out=P, in0=ps_f, in1=alpha[0:TAG, :], op=AL.mult)
            ps_c = psumg.tile([1, BL], f32, name="ps_c", tag="ps_c")
            nc.tensor.matmul(ps_c, ones52, P, start=True, stop=True)
            nc.vector.tensor_copy(stage_sc[0:1, 0:BL], ps_c)
            nc.sync.dma_start(out=out_scan[:, :], in_=stage_sc)

            # ================= gold tail: transition contractions =============
            scrap2 = small.tile([TAG, TAG], f32, name="scrap2", tag="scrap2")
            nc.vector.tensor_tensor(
                out=scrap2,
                in0=SM[0:TAG, C_CP : C_CP + TAG],
                in1=tr_sb,
                op=AL.mult,
            )
            nc.vector.tensor_reduce(
                out=stage_gold[0:TAG, 1:2],
                in_=scrap2,
                axis=mybir.AxisListType.X,
                op=AL.add,
            )
            nc.vector.tensor_tensor(
                out=stage_gold[0:TAG, 2:3],
                in0=SM[0:TAG, C_CE : C_CE + 1],
                in1=tr_sb[:, STOP : STOP + 1],
                op=AL.mult,
            )
            nc.sync.dma_start(out=out_gold[:, :], in_=stage_gold)

    nc.compile()
    return nc


def _prep_core_inputs(feats, transitions, mask, tags, core):
    """Host marshalling of the core's batch shard: layout + integer prep."""
    import ml_dtypes

    f32 = np.float32
    bf = ml_dtypes.bfloat16
    sl = slice(core * BL, (core + 1) * BL)
    f = np.ascontiguousarray(feats[sl]).astype(f32, copy=False)   # (BL,S,T)
    m = mask[sl].astype(f32)                                      # (BL,S)
    tg = tags[sl].astype(np.int64)                                # (BL,S)

    fT = f.transpose(2, 1, 0)                                     # (T,S,BL)
    ft2 = np.zeros((128, HALF, BL), f32)
    ft2[0:TAG, 0:127, :] = fT[:, 1:128, :]
    ft2[STOP, 0:127, :] = 0.0
    ft2[0:TAG, 127, :] = -200.0                                   # dead fwd slot
    ft2[ROWB : ROWB + TAG, :, :] = fT[:, 255:127:-1, :]
    ft2[ROWB + STOP, :, :] = 0.0

    mtb = np.ascontiguousarray(m.T[255:127:-1, :])                # (HALF,BL)
    mg = np.stack([mtb * MGATE, (1.0 - mtb) * MGATE]).astype(f32)  # (2,HALF,BL)

    smalls = np.zeros((128, SMW), f32)
    trp = transitions.astype(f32).copy()
    trp[STOP, STOP] = 0.0
    smalls[0:TAG, C_TR : C_TR + TAG] = trp
    smalls[0:TAG, C_TT : C_TT + TAG] = trp.T
    # colcs: col0 unused, col1 = per-partition exp bias
    smalls[0:TAG, C_CC + 1] = -C0
    smalls[STOP, C_CC + 1] = -MGATE
    smalls[ROWB : ROWB + TAG, C_CC + 1] = -(MGATE + C0)
    smalls[ROWB + STOP, C_CC + 1] = -MGATE   # pairs with the (1-m) gate row
    smalls[0:TAG, C_HS] = 1.0                 # Hsum col0: fwd half
    smalls[ROWB : ROWB + TAG, C_HS + 1] = 1.0
    smalls[0, C_HB : C_HB + TAG] = 1.0        # Hbc row0 -> fwd rows
    smalls[1, C_HB + ROWB : C_HB + ROWB + TAG] = 1.0
    f0 = f[:, 0, :].T.copy()
    f0[STOP] = 0.0
    smalls[0:TAG, C_F0 : C_F0 + BL] = f0

    prev = np.concatenate([np.full((BL, 1), START, np.int64), tg[:, :-1]], axis=1)
    msk = m > 0
    cntp = np.zeros((TAG, TAG), f32)
    np.add.at(cntp, (prev[msk], tg[msk]), 1.0)
    smalls[0:TAG, C_CP : C_CP + TAG] = cntp
    lengths = m.astype(np.int64).sum(axis=1)
    end_ids = np.take_along_axis(tg, (lengths - 1)[:, None], axis=1)[:, 0]
    cnte = np.zeros((TAG,), f32)
    np.add.at(cnte, end_ids, 1.0)
    smalls[0:TAG, C_CE] = cnte

    featsN = np.ascontiguousarray(f.reshape(BL * S, TAG)).reshape(128, M32, TAG)
    tags_m = np.where(m > 0, tg, -1)
    yhot = (
        (tags_m.reshape(BL * S, 1) == np.arange(TAG)[None, :])
        .astype(f32)
        .reshape(128, M32, TAG)
    )
    fny = np.concatenate([featsN, yhot], axis=2)                  # (128,M32,104)

    return {
        "featsT2": ft2,
        "mgate": mg,
        "smalls": smalls,
        "onebl": np.ones((1, BL), bf),
        "fny": np.ascontiguousarray(fny),
    }


def _combine(results, mask):
    """Host-side unshard: logs of staged scales + partial sums -> scalar."""
    lengths = np.asarray(mask).astype(np.int64).sum(axis=1)       # (B,)
    fwd = np.float64(0.0)
    gold = np.float64(0.0)
    for core, res in enumerate(results):
        sc = res["out_scan"].astype(np.float64)                   # (2, 3*BL)
        gl = res["out_gold"].astype(np.float64)                   # (128, 4)
        ln = (
            np.log(sc[0, 0:BL])
            + np.log(sc[0, BL : 2 * BL])
            + np.log(sc[1, BL : 2 * BL])
        )
        lens = lengths[core * BL : (core + 1) * BL].astype(np.float64)
        fwd += (ln + C0 * lens).sum()
        gold += gl[0, 0] + gl[0:TAG, 1].sum() + gl[0:TAG, 2].sum()
    return np.asarray(fwd - gold, dtype=np.float32)[()]


def kernel(feats, transitions, mask, tags):
    feats = np.asarray(feats)
    transitions = np.asarray(transitions)
    mask = np.asarray(mask)
    tags = np.asarray(tags)

    if "nc" not in _CACHE:
        _CACHE["nc"] = _build_nc(debug=False)
    nc = _CACHE["nc"]

    from concourse import bass_utils

    in_maps = [
        _prep_core_inputs(feats, transitions, mask, tags, c) for c in range(NCORES)
    ]
    out = bass_utils.run_bass_kernel_spmd(nc, in_maps, core_ids=list(range(NCORES)))
    return _combine(out.results, mask)


# revision 23
# speedup vs baseline: 2.1625x; 1.0091x over previous
"""Trainium2 Bass kernel for CRF negative log-likelihood (nn_CRF) — v3.

Strategy:
  - data-parallel over batch: 8 cores x 16 sequences each.
  - forward algorithm in the exp domain: the fwd chain (alpha, t=0..127) and
    the bwd chain (beta, t=255..128) are MERGED into one 128-step scan over a
    block-diagonal bf16 stationary G (Etil at rows/cols 0..51, Etil^T at
    64..115; blocks at 0/64 keep engine partition bases 32-aligned).  Each
    step is ONE bf16 matmul (PE) + ONE elementwise multiply (DVE); the
    serial PE->DVE->PE round trip (~435ns) is the latency floor, so all other
    work lives on ACT/GPSIMD/DMA:
      * emissions D2 (128, HALF, BL) bf16: fwd emissions on rows 0..51, the
        time-reversed bwd emissions on rows 64..115; exp(-C0) rescale and the
        absorbing-STOP mask gate folded in via per-partition ACT bias plus
        DMA accumulate (accum_op=add) of host-scaled mask rows — no vector
        engine involvement at all.
      * gold-score emission gather: host-built one-hot * feats on GPSIMD.
      * host does integer prep only (one-hots, pair/end counts); all float
        math on feats/transitions happens on device.
  - one renorm at k=64 bounds fp32/bf16 range; raw colsums staged out and the
    host adds the logs back (plus C0 * length per sequence).
"""

import numpy as np

TAG = 52
START, STOP = TAG - 2, TAG - 1
B, S = 128, 256
NCORES = 8
BL = B // NCORES            # 16 sequences per core
HALF = S // 2               # 128 steps per direction
C0 = 4.9                    # constant per-step rescale (nats)
MGATE = 64.0                # mask gate constant (exp(-64) == 0 in fp32)
M32 = (S * BL) // 128       # 32 free rows for the (128, M32, TAG) gold layout
ROWB = 64                   # partition offset of the bwd block
GROUPS = ((0, 16), (16, 48), (48, HALF))   # emission build chunk groups

# packed "smalls" layout (columns in a single (128, SMW) f32 tensor)
C_TR = 0            # [0:52]   rows 0:52  transitions (STOP,STOP pre-patched)
C_TT = 52           # [52:104] rows 0:52  transitions TRANSPOSED (same patch)
C_CC = 104          # [104:106]           colcs (sgate unused now, bias)
C_HS = 106          # [106:108]           Hsum pattern (f32 -> bf16 copy)
C_HB = 108          # [108:236] rows 0:2  Hbc
C_F0 = 236          # [236:252] rows 0:52 feats[:, 0, :].T
C_CP = 252          # [252:304] rows 0:52 pair counts
C_CE = 304          # [304:305] rows 0:52 end counts
C_B0 = 305          # [305:306] rows 0:52 init bias (top bias + trans[START,:])
SMW = 306

_CACHE: dict = {}


def _build_nc(debug: bool = False):
    import concourse.bass as bass
    import concourse.mybir as mybir
    import concourse.tile as tile
    from concourse import bacc

    f32 = mybir.dt.float32
    bf16 = mybir.dt.bfloat16
    AL = mybir.AluOpType
    EXP = mybir.ActivationFunctionType.Exp

    nc = bacc.Bacc("TRN2", target_bir_lowering=False, debug=debug)

    # ---- external inputs (per-core shards, host-marshalled layouts) ----
    featsT2 = nc.dram_tensor("featsT2", (128, HALF, BL), f32, kind="ExternalInput")
    mgate = nc.dram_tensor("mgate", (2, HALF, BL), f32, kind="ExternalInput")
    smalls = nc.dram_tensor("smalls", (128, SMW), f32, kind="ExternalInput")
    onebl = nc.dram_tensor("onebl", (1, BL), bf16, kind="ExternalInput")
    fny = nc.dram_tensor("fny", (128, M32, 2 * TAG), f32, kind="ExternalInput")

    # ---- external outputs ----
    # out_scan: [0, 0:BL] = midpoint colsum; [0/1, BL:2BL] = fwd/bwd renorm sums
    out_scan = nc.dram_tensor("out_scan", (2, 3 * BL), f32, kind="ExternalOutput")
    # out_gold: [0,0] = emit sum; col1 = trans*cnt partials; col2 = end partials
    out_gold = nc.dram_tensor("out_gold", (128, 4), f32, kind="ExternalOutput")

    with tile.TileContext(nc) as tc:
        with (
            tc.tile_pool(name="persist", bufs=1) as persist,
            tc.tile_pool(name="chunks", bufs=1) as chunks,
            tc.tile_pool(name="state", bufs=3) as statep,
            tc.tile_pool(name="small", bufs=2) as small,
            tc.tile_pool(name="gold", bufs=1) as goldp,
            tc.tile_pool(name="psum", bufs=1, space="PSUM") as psum,
            tc.tile_pool(name="psumg", bufs=1, space="PSUM") as psumg,
        ):
            # ---- ACT activation-table prefetch: dummy exp at t=0 ----
            junk = small.tile([1, 1], f32, name="junk", tag="junk")
            nc.gpsimd.memset(junk, 0.0)
            junk2 = small.tile([1, 1], f32, name="junk2", tag="junk2")
            nc.scalar.activation(out=junk2, in_=junk, func=EXP)

            # ---- emission group-0 DMAs first: longest dependency chain ----
            fts = {}
            for s0, s1 in GROUPS:
                fts[s0] = chunks.tile(
                    [128, s1 - s0, BL], f32, name=f"ft{s0}", tag=f"ft{s0}"
                )
            n0 = GROUPS[0][1]
            nc.sync.dma_start(out=fts[0], in_=featsT2[:, 0:n0, :])
            # group-0 mask rows for the DVE STT gate (broadcast along tags)
            mrep0 = chunks.tile([128, n0, BL], f32, name="mrep0", tag="mrep0")
            src0 = bass.AP(tensor=mgate, offset=0, ap=[[0, TAG], [BL, n0], [1, BL]])
            nc.sync.dma_start(out=mrep0[ROWB : ROWB + TAG, :, :], in_=src0)
            srcn0 = bass.AP(
                tensor=mgate, offset=HALF * BL, ap=[[0, 1], [BL, n0], [1, BL]]
            )
            nc.sync.dma_start(
                out=mrep0[ROWB + TAG - 1 : ROWB + TAG, :, :], in_=srcn0
            )

            # ================= packed smalls =================
            SM = persist.tile([128, SMW], f32, name="SM", tag="SM")
            nc.sync.dma_start(out=SM, in_=smalls[:, :])
            tr_sb = SM[0:TAG, C_TR : C_TR + TAG]
            colc = SM[:, C_CC : C_CC + 2]
            Hbc = SM[0:2, C_HB : C_HB + 128]

            # ================= scan state init =================
            # V0 top = exp(f0 + bias + trans[START, :]) (bias col C_B0);
            # bottom = onehot(STOP) via tiny DMA (arbitrary partition base)
            V = statep.tile([128, BL], bf16, name="V0", tag="V")
            nc.vector.memset(V, 0.0)
            nc.scalar.activation(
                out=V[0:TAG, :],
                in_=SM[0:TAG, C_F0 : C_F0 + BL],
                func=EXP,
                bias=SM[0:TAG, C_B0 : C_B0 + 1],
            )
            nc.sync.dma_start(
                out=V[ROWB + STOP : ROWB + STOP + 1, :], in_=onebl[:, :]
            )

            # ================= transitions -> G blockdiag (direct ACT) =======
            G = persist.tile([128, 128], bf16, name="G", tag="G")
            nc.vector.memset(G, 0.0)
            nc.scalar.activation(
                out=G[0:TAG, 0:TAG], in_=SM[0:TAG, C_TR : C_TR + TAG], func=EXP
            )
            nc.scalar.activation(
                out=G[ROWB : ROWB + TAG, ROWB : ROWB + TAG],
                in_=SM[ROWB : ROWB + TAG, C_TT : C_TT + TAG],
                func=EXP,
            )
            G2 = persist.tile([128, TAG], bf16, name="G2", tag="G2")
            nc.vector.memset(G2, 0.0)
            nc.scalar.activation(
                out=G2[ROWB : ROWB + TAG, :],
                in_=SM[ROWB : ROWB + TAG, C_TT : C_TT + TAG],
                func=EXP,
            )

            Hsum = persist.tile([128, 2], bf16, name="Hsum", tag="Hsum")
            nc.vector.tensor_copy(Hsum, SM[:, C_HS : C_HS + 2])
            ones52 = persist.tile([TAG, 1], bf16, name="ones52", tag="ones52")
            nc.vector.memset(ones52, 1.0)

            # ================= emission tensor D2 (128, HALF, BL) ============
            D2 = persist.tile([128, HALF, BL], bf16, name="D2", tag="D2")
            nc.vector.memset(D2, 0.0)

            def emit_exps(s0, s1):
                ft = fts[s0]
                nc.scalar.activation(
                    out=D2[0:TAG, s0:s1, :],
                    in_=ft[0:TAG, :, :],
                    func=EXP,
                    bias=colc[0:TAG, 1:2],
                )
                nc.scalar.activation(
                    out=D2[ROWB : ROWB + TAG, s0:s1, :],
                    in_=ft[ROWB : ROWB + TAG, :, :],
                    func=EXP,
                    bias=colc[ROWB : ROWB + TAG, 1:2],
                )

            # group 0: DVE STT gate (DVE is idle pre-scan); the gate adds
            # m*MGATE on rows 64..114 and (1-m)*MGATE on row 115, all staged
            # in mrep0 by the two broadcast DMAs above.
            nc.vector.tensor_tensor(
                out=fts[0][ROWB : ROWB + TAG, :, :],
                in0=mrep0[ROWB : ROWB + TAG, :, :],
                in1=fts[0][ROWB : ROWB + TAG, :, :],
                op=AL.add,
            )
            emit_exps(*GROUPS[0])

            def emit_gate(s0, s1):
                # mask gate via DMA accumulate: rows 64..114 += m*MGATE,
                # row 115 (STOP) += (1-m)*MGATE
                n = s1 - s0
                ft = fts[s0]
                srcp = bass.AP(
                    tensor=mgate,
                    offset=s0 * BL,
                    ap=[[0, TAG - 1], [BL, n], [1, BL]],
                )
                nc.gpsimd.dma_start(
                    out=ft[ROWB : ROWB + TAG - 1, :, :], in_=srcp, accum_op=AL.add
                )
                srcn = bass.AP(
                    tensor=mgate,
                    offset=HALF * BL + s0 * BL,
                    ap=[[0, 1], [BL, n], [1, BL]],
                )
                nc.gpsimd.dma_start(
                    out=ft[ROWB + TAG - 1 : ROWB + TAG, :, :],
                    in_=srcn,
                    accum_op=AL.add,
                )

            for s0, s1 in GROUPS[1:]:
                nc.sync.dma_start(out=fts[s0], in_=featsT2[:, s0:s1, :])
                emit_gate(s0, s1)
                emit_exps(s0, s1)

            # ================= interleaved fwd/bwd scan =================
            stage_sc = persist.tile([2, 3 * BL], f32, name="stage_sc", tag="ssc")
            nc.vector.memset(stage_sc, 0.0)

            alpha = None
            for k in range(1, HALF + 1):
                ps = psum.tile([128, BL], f32, name="ps", tag="ps", bufs=2)
                nc.tensor.matmul(ps, G, V, start=True, stop=True)
                Vn = statep.tile([128, BL], bf16, name="Vn", tag="V")
                nc.vector.tensor_tensor(
                    out=Vn, in0=ps, in1=D2[:, k - 1, :], op=AL.mult
                )
                V = Vn
                if k == 64:
                    ps_s = psum.tile([2, BL], f32, name="ps_s", tag="ps_s")
                    nc.tensor.matmul(ps_s, Hsum, V, start=True, stop=True)
                    nc.vector.tensor_copy(stage_sc[:, BL : 2 * BL], ps_s)
                    rcp = small.tile([2, BL], f32, name="rcp", tag="rcp")
                    nc.vector.reciprocal(rcp, ps_s)
                    ps_b = psum.tile([128, BL], f32, name="ps_b", tag="ps_b")
                    nc.tensor.matmul(ps_b, Hbc, rcp, start=True, stop=True)
                    Vr = statep.tile([128, BL], bf16, name="Vr", tag="V")
                    nc.vector.tensor_tensor(out=Vr, in0=ps_b, in1=V, op=AL.mult)
                    V = Vr
                if k == HALF - 1:
                    alpha = V

            # ================= gold (GPSIMD, off the scan path) ==============
            FNY = goldp.tile([128, M32, 2 * TAG], f32, name="FNY", tag="FNY")
            nc.sync.dma_start(out=FNY, in_=fny[:, :, :])
            stage_gold = goldp.tile([128, 4], f32, name="stage_gold", tag="sg")
            nc.gpsimd.memset(stage_gold, 0.0)
            scrap = goldp.tile([128, M32, TAG], f32, name="scrap", tag="scrap")
            nc.gpsimd.tensor_tensor(
                out=scrap,
                in0=FNY[:, :, 0:TAG],
                in1=FNY[:, :, TAG : 2 * TAG],
                op=AL.mult,
            )
            nc.gpsimd.tensor_reduce(
                out=stage_gold[0:1, 0:1],
                in_=scrap,
                axis=mybir.AxisListType.XYZWC,
                op=AL.add,
            )

            # ================= tail: Z = alpha_127 . beta_127 =================
            ps_f = psumg.tile([TAG, BL], f32, name="ps_f", tag="ps_f")
            nc.tensor.matmul(ps_f, G2, V, start=True, stop=True)
            P = small.tile([TAG, BL], bf16, name="P", tag="P")
            nc.vector.tensor_tensor(out=P, in0=ps_f, in1=alpha[0:TAG, :], op=AL.mult)
            ps_c = psumg.tile([1, BL], f32, name="ps_c", tag="ps_c")
            nc.tensor.matmul(ps_c, ones52, P, start=True, stop=True)
            nc.vector.tensor_copy(stage_sc[0:1, 0:BL], ps_c)
            nc.sync.dma_start(out=out_scan[:, :], in_=stage_sc)

            # ================= gold tail: transition contractions =============
            scrap2 = small.tile([TAG, TAG], f32, name="scrap2", tag="scrap2")
            nc.vector.tensor_tensor(
                out=scrap2,
                in0=SM[0:TAG, C_CP : C_CP + TAG],
                in1=tr_sb,
                op=AL.mult,
            )
            nc.vector.tensor_reduce(
                out=stage_gold[0:TAG, 1:2],
                in_=scrap2,
                axis=mybir.AxisListType.X,
                op=AL.add,
            )
            nc.vector.tensor_tensor(
                out=stage_gold[0:TAG, 2:3],
                in0=SM[0:TAG, C_CE : C_CE + 1],
                in1=tr_sb[:, STOP : STOP + 1],
                op=AL.mult,
            )
            nc.sync.dma_start(out=out_gold[:, :], in_=stage_gold)

    nc.compile()
    return nc


def _prep_core_inputs(feats, transitions, mask, tags, core):
    """Host marshalling of the core's batch shard: layout + integer prep."""
    import ml_dtypes

    f32 = np.float32
    bf = ml_dtypes.bfloat16
    sl = slice(core * BL, (core + 1) * BL)
    f = np.ascontiguousarray(feats[sl]).astype(f32, copy=False)   # (BL,S,T)
    m = mask[sl].astype(f32)                                      # (BL,S)
    tg = tags[sl].astype(np.int64)                                # (BL,S)

    fT = f.transpose(2, 1, 0)                                     # (T,S,BL)
    ft2 = np.zeros((128, HALF, BL), f32)
    ft2[0:TAG, 0:127, :] = fT[:, 1:128, :]
    ft2[STOP, 0:127, :] = 0.0
    ft2[0:TAG, 127, :] = -200.0                                   # dead fwd slot
    ft2[ROWB : ROWB + TAG, :, :] = fT[:, 255:127:-1, :]
    ft2[ROWB + STOP, :, :] = 0.0

    mtb = np.ascontiguousarray(m.T[255:127:-1, :])                # (HALF,BL)
    mg = np.stack([mtb * MGATE, (1.0 - mtb) * MGATE]).astype(f32)  # (2,HALF,BL)

    smalls = np.zeros((128, SMW), f32)
    trp = transitions.astype(f32).copy()
    trp[STOP, STOP] = 0.0
    smalls[0:TAG, C_TR : C_TR + TAG] = trp
    smalls[ROWB : ROWB + TAG, C_TT : C_TT + TAG] = trp.T   # rows 64:116: ACT
    # reads/writes must share a 32-aligned partition base with their output
    # colcs: col0 unused, col1 = per-partition exp bias
    smalls[0:TAG, C_CC + 1] = -C0
    smalls[STOP, C_CC + 1] = -MGATE
    smalls[ROWB : ROWB + TAG, C_CC + 1] = -(MGATE + C0)
    smalls[ROWB + STOP, C_CC + 1] = -MGATE   # pairs with the (1-m) gate row
    smalls[0:TAG, C_HS] = 1.0                 # Hsum col0: fwd half
    smalls[ROWB : ROWB + TAG, C_HS + 1] = 1.0
    smalls[0, C_HB : C_HB + TAG] = 1.0        # Hbc row0 -> fwd rows
    smalls[1, C_HB + ROWB : C_HB + ROWB + TAG] = 1.0
    f0 = f[:, 0, :].T.copy()
    f0[STOP] = 0.0
    smalls[0:TAG, C_F0 : C_F0 + BL] = f0
    # init bias: top exp bias + trans[START, :] (folds the alpha_0 init
    # multiply by exp(trans[START, j]) into the one ACT that builds V0)
    smalls[0:TAG, C_B0] = smalls[0:TAG, C_CC + 1] + trp[START, :]

    prev = np.concatenate([np.full((BL, 1), START, np.int64), tg[:, :-1]], axis=1)
    msk = m > 0
    cntp = np.zeros((TAG, TAG), f32)
    np.add.at(cntp, (prev[msk], tg[msk]), 1.0)
    smalls[0:TAG, C_CP : C_CP + TAG] = cntp
    lengths = m.astype(np.int64).sum(axis=1)
    end_ids = np.take_along_axis(tg, (lengths - 1)[:, None], axis=1)[:, 0]
    cnte = np.zeros((TAG,), f32)
    np.add.at(cnte, end_ids, 1.0)
    smalls[0:TAG, C_CE] = cnte

    featsN = np.ascontiguousarray(f.reshape(BL * S, TAG)).reshape(128, M32, TAG)
    tags_m = np.where(m > 0, tg, -1)
    yhot = (
        (tags_m.reshape(BL * S, 1) == np.arange(TAG)[None, :])
        .astype(f32)
        .reshape(128, M32, TAG)
    )
    fny = np.concatenate([featsN, yhot], axis=2)                  # (128,M32,104)

    return {
        "featsT2": ft2,
        "mgate": mg,
        "smalls": smalls,
        "onebl": np.ones((1, BL), bf),
        "fny": np.ascontiguousarray(fny),
    }


def _combine(results, mask):
    """Host-side unshard: logs of staged scales + partial sums -> scalar."""
    lengths = np.asarray(mask).astype(np.int64).sum(axis=1)       # (B,)
    fwd = np.float64(0.0)
    gold = np.float64(0.0)
    for core, res in enumerate(results):
        sc = res["out_scan"].astype(np.float64)                   # (2, 3*BL)
        gl = res["out_gold"].astype(np.float64)                   # (128, 4)
        ln = (
            np.log(sc[0, 0:BL])
            + np.log(sc[0, BL : 2 * BL])
            + np.log(sc[1, BL : 2 * BL])
        )
        lens = lengths[core * BL : (core + 1) * BL].astype(np.float64)
        fwd += (ln + C0 * lens).sum()
        gold += gl[0, 0] + gl[0:TAG, 1].sum() + gl[0:TAG, 2].sum()
    return np.asarray(fwd - gold, dtype=np.float32)[()]


def kernel(feats, transitions, mask, tags):
    feats = np.asarray(feats)
    transitions = np.asarray(transitions)
    mask = np.asarray(mask)
    tags = np.asarray(tags)

    if "nc" not in _CACHE:
        _CACHE["nc"] = _build_nc(debug=False)
    nc = _CACHE["nc"]

    from concourse import bass_utils

    in_maps = [
        _prep_core_inputs(feats, transitions, mask, tags, c) for c in range(NCORES)
    ]
    out = bass_utils.run_bass_kernel_spmd(nc, in_maps, core_ids=list(range(NCORES)))
    return _combine(out.results, mask)
